# revision 21
# baseline (speedup 1.0000x reference)
"""Trainium2 Bass kernel for CellLineSpecificLinearModule (routed per-expert linear).

out_b = W[cell_line_b] @ x_b + b[cell_line_b]

Sharding: expert-parallel over 8 NeuronCores. Core c owns experts
[8c, 8c+8). The host routes samples by cell_line, packs each core's
samples grouped by expert (padded to a common per-expert capacity S so
the SPMD program is identical across cores), pre-transposes the expert
matrices to [fin, fout] and swizzles both operands into the exact SBUF
partition layout, so every device DMA is a contiguous 128-partition
transfer. On device, per expert: 4 K-tile matmuls accumulate
x_seg @ W_e^T into PSUM and a rank-1 (ones x bias) matmul adds the
bias; PSUM is copied to SBUF and DMA'd out. The host scatters result
rows back to the original sample order.

Total HBM traffic per core ~ 8MB of expert weights (the fleet-wide
minimum: each expert matrix is read exactly once) + ~1MB of x/out.
"""

import os

_p = os.environ.get("JAX_PLATFORMS", "")
if _p and "axon" not in _p.split(","):
    os.environ["JAX_PLATFORMS"] = "axon," + _p

import numpy as np

import concourse.bacc as bacc
import concourse.mybir as mybir
import concourse.tile as tile
from concourse.bass_utils import run_bass_kernel_spmd

N_CORES = 8
N_CL = 64
E_PER_CORE = N_CL // N_CORES  # 8 experts per core
F_IN = 512
F_OUT = 512
KT = F_IN // 128  # 4 K-tiles of 128
B = 1024

_prog_cache: dict = {}

# test-harness hooks (ignored in grading: defaults give a plain run)
TRACE = False
LAST_RESULTS = None


def _build_program(S: int, reps: int = 1, loop_n: int | None = None):
    """Build + compile the SPMD program for per-expert capacity S.

    reps>1 repeats the whole computation unrolled; loop_n wraps the body
    in a hardware For_i loop (benchmark variants: slope of wall-time vs
    iteration count isolates device time from dispatch overhead).
    """
    key = (S, reps, loop_n)
    if key in _prog_cache:
        return _prog_cache[key]

    dt = mybir.dt.float32
    XCOLS = E_PER_CORE * S  # packed sample columns per core

    # SBUF budget per partition (~192KB): x 128*S B, w 8KB/buf, out 2KB/buf
    if S <= 128:
        bufs_w, bufs_o = 8, 4
    elif S <= 512:
        bufs_w, bufs_o = 4, 3
    else:
        bufs_w, bufs_o = 2, 2

    nc = bacc.Bacc(
        "TRN2",
        target_bir_lowering=False,
        debug=False,
        enable_asserts=False,
        num_devices=N_CORES,
    )

    dtr = mybir.dt.float32r  # 4-byte fp32 payload, PE streams at 1 cyc/row
    xs = nc.dram_tensor("xs", [128, KT * XCOLS], dtr, kind="ExternalInput")
    wt = nc.dram_tensor("wt", [E_PER_CORE, 128, KT * F_OUT], dtr, kind="ExternalInput")
    bs = nc.dram_tensor("bs", [1, E_PER_CORE * F_OUT], dtr, kind="ExternalInput")
    on = nc.dram_tensor("on", [1, 128], dtr, kind="ExternalInput")
    out = nc.dram_tensor("out", [E_PER_CORE * S, F_OUT], dt, kind="ExternalOutput")

    with tile.TileContext(nc) as tc:
        with (
            tc.tile_pool(name="xp", bufs=1 if reps == 1 else 2) as xp,
            tc.tile_pool(name="wp", bufs=bufs_w) as wp,
            tc.tile_pool(name="op", bufs=bufs_o) as op,
            tc.tile_pool(name="cp", bufs=1) as cp,
            tc.tile_pool(name="ps", bufs=8, space="PSUM") as ps,
        ):
            ones_t = cp.tile([1, 128], dtr, tag="ones")
            nc.scalar.dma_start(ones_t[:], on[:])

            # sync ring: pure W stream (the 8MB/core bottleneck, ~429GB/s).
            # scalar ring: x, bias, output writes (~1MB total).
            def body():
                for _rep in range(reps):
                    x_t = xp.tile([128, KT * XCOLS], dtr, tag="x")
                    nc.scalar.dma_start(x_t[:], xs[:])
                    b_t = cp.tile([1, E_PER_CORE * F_OUT], dtr, tag="b")
                    nc.scalar.dma_start(b_t[:], bs[:])

                    for e in range(E_PER_CORE):
                        w_t = wp.tile([128, KT * F_OUT], dtr, tag="w")
                        nc.sync.dma_start(w_t[:], wt[e])
                        for m0 in range(0, S, 128):
                            mm = min(128, S - m0)
                            acc = ps.tile([mm, F_OUT], dt, tag="acc")
                            for k in range(KT):
                                nc.tensor.matmul(
                                    acc[:],
                                    x_t[
                                        :,
                                        k * XCOLS + e * S + m0 : k * XCOLS
                                        + e * S
                                        + m0
                                        + mm,
                                    ],
                                    w_t[:, k * F_OUT : (k + 1) * F_OUT],
                                    start=(k == 0),
                                    stop=False,
                                )
                            nc.tensor.matmul(
                                acc[:],
                                ones_t[0:1, 0:mm],
                                b_t[0:1, e * F_OUT : (e + 1) * F_OUT],
                                start=False,
                                stop=True,
                            )
                            o_t = op.tile([mm, F_OUT], dt, tag="o")
                            nc.vector.tensor_copy(o_t[:], acc[:])
                            nc.scalar.dma_start(
                                out[e * S + m0 : e * S + m0 + mm, :], o_t[:]
                            )

            if loop_n is not None:
                with tc.For_i(0, loop_n, 1):
                    body()
            else:
                body()

    nc.compile()
    _prog_cache[key] = nc
    return nc


def _swizzle_kp(a: np.ndarray) -> np.ndarray:
    """[..., KT*128, N] -> [..., 128, KT*N]: partition p free-block k holds row k*128+p."""
    lead = a.shape[:-2]
    n = a.shape[-1]
    return (
        a.reshape(*lead, KT, 128, n)
        .swapaxes(-3, -2)
        .reshape(*lead, 128, KT * n)
        .copy()
    )


def _prepare(x, cell_line, cell_line_matrices, cell_line_offsets):
    """Host-side routing + packing. Returns (S, in_maps, groups)."""
    x_np = np.ascontiguousarray(np.asarray(x, dtype=np.float32))
    cl = np.asarray(cell_line).astype(np.int64).ravel()
    W = np.asarray(cell_line_matrices, dtype=np.float32)
    bvec = np.asarray(cell_line_offsets, dtype=np.float32)[:, :, 0]  # [64, 512]

    assert x_np.shape == (B, F_IN) and W.shape == (N_CL, F_OUT, F_IN)
    assert cl.shape == (B,) and cl.min() >= 0 and cl.max() < N_CL

    # --- host-side routing (the expert-parallel "all-to-all") ---
    counts = np.bincount(cl, minlength=N_CL)
    S = max(int(counts.max()), 4)
    S = (S + 3) // 4 * 4
    order = np.argsort(cl, kind="stable")
    starts = np.zeros(N_CL + 1, dtype=np.int64)
    starts[1:] = np.cumsum(counts)
    groups = [order[starts[g] : starts[g + 1]] for g in range(N_CL)]

    # weights: W[g].T -> [fin, fout], swizzled to [128, KT*F_OUT]
    w_sw = _swizzle_kp(W.transpose(0, 2, 1))  # [64, 128, KT*F_OUT]

    in_maps = []
    for c in range(N_CORES):
        xpacked = np.zeros((E_PER_CORE * S, F_IN), dtype=np.float32)
        for j in range(E_PER_CORE):
            idx = groups[c * E_PER_CORE + j]
            if len(idx):
                xpacked[j * S : j * S + len(idx)] = x_np[idx]
        in_maps.append(
            {
                "xs": _swizzle_kp(xpacked.T),  # [128, KT*8S]
                "wt": np.ascontiguousarray(w_sw[c * E_PER_CORE : (c + 1) * E_PER_CORE]),
                "bs": np.ascontiguousarray(
                    bvec[c * E_PER_CORE : (c + 1) * E_PER_CORE].reshape(1, -1)
                ),
                "on": np.ones((1, 128), dtype=np.float32),
            }
        )
    return S, in_maps, groups


def kernel(x, cell_line, cell_line_matrices, cell_line_offsets):
    S, in_maps, groups = _prepare(x, cell_line, cell_line_matrices, cell_line_offsets)

    nc = _build_program(S)
    res = run_bass_kernel_spmd(nc, in_maps, list(range(N_CORES)), trace=TRACE)
    global LAST_RESULTS
    LAST_RESULTS = res

    # --- gather/unshard ---
    out_full = np.empty((B, F_OUT), dtype=np.float32)
    for c in range(N_CORES):
        oc = res.results[c]["out"]
        for j in range(E_PER_CORE):
            idx = groups[c * E_PER_CORE + j]
            if len(idx):
                out_full[idx] = oc[j * S : j * S + len(idx)]

    return (out_full, np.asarray(cell_line))
